# revision 42
# baseline (speedup 1.0000x reference)
"""Differentiable K-Means layer (vq_codebook) TRN2 kernel.

Strategy (8 NeuronCores, data-parallel over N = B*S = 32768 rows):
  - shard x row-wise: core i gets x[i] (B=8 -> one batch element per core).
  - host splits x into a bf16 hi/lo pair (exact to ~17 mantissa bits); the
    device reconstitutes f32r (TRN2's full-rate 12-mantissa-bit fp32 matmul
    dtype). The bf16 halves ride the DMA xbar transpose (bf16-only) to build
    x^T tiles for the distance GEMM with zero TensorE transpose cost.
  - host also pre-computes c^T and -|c|^2/2 in f32r (pure c-preprocessing).
  - cross = x @ c^T in f32r at full PE rate, fused -|c|^2/2 rank-1 update
    (so PSUM holds the exact logits/(-2/T)), exact row-max on DVE, exp on
    ACT with fused row-sum accumulation, normalize on DVE.
  - weighted sums ws = a^T @ x and sum_w = 1^T a accumulate in PSUM across
    all 32 row tiles, then one 1.05 MB AllReduce across the 8 cores.
  - centroid repulsion (K x K) is computed redundantly on every core during
    the pipeline ramp (distance/weight phase before the first exp so the
    ACT spline table switches once); the momentum/centroid update is
    redundant on every core; core 0's copy is returned.
  - emission is software-pipelined (ws MMs for tile n are emitted after the
    cross MMs for tile n+1) so the TensorE static schedule never
    head-of-line blocks on the softmax chain.
Outputs: assignments (B,S,K) sharded back, centroids_new, mom_new.
"""

import numpy as np
import ml_dtypes

B, S, D, K = 8, 4096, 512, 512
N_CORES = 8
NLOC = (B * S) // N_CORES          # 4096 rows per core
NT = NLOC // 128                   # 32 row tiles per core
GROUP = 512                        # rows per transpose group
NGRP = NLOC // GROUP               # 8 groups
TPG = GROUP // 128                 # 4 row tiles per group
DC = D // 128                      # 4 contraction chunks
KC = K // 128                      # 4 centroid chunks

EPS = 1e-7
TEMP = 0.1
MOMENTUM = 0.9
CENTROID_LR = 0.1
REP_STRENGTH = 0.1

_CACHE = {}


def _build_nc():
    import os as _os
    import concourse.bacc as bacc
    import concourse.mybir as mybir
    import concourse.tile as tile

    f32 = mybir.dt.float32
    f32r = mybir.dt.float32r
    bf16 = mybir.dt.bfloat16
    AF = mybir.ActivationFunctionType
    AX = mybir.AxisListType
    OP = mybir.AluOpType

    TLSIM = bool(int(_os.environ.get("KMEANS_TLSIM", "0")))
    DBG = bool(int(_os.environ.get("KMEANS_DBG", "0")))

    nc = bacc.Bacc("TRN2", target_bir_lowering=False, debug=False,
                   num_devices=N_CORES)

    xh_d = nc.dram_tensor("xh", [NLOC, D], bf16, kind="ExternalInput")
    xl_d = nc.dram_tensor("xl", [NLOC, D], bf16, kind="ExternalInput")
    c_d = nc.dram_tensor("c", [K, D], f32, kind="ExternalInput")
    cT_d = nc.dram_tensor("cT", [D, K], f32r, kind="ExternalInput")
    nhcsq_d = nc.dram_tensor("nhcsq", [1, K], f32r, kind="ExternalInput")
    mom_d = nc.dram_tensor("mom", [K, D], f32, kind="ExternalInput")
    a_d = nc.dram_tensor("a_out", [NLOC, K], f32, kind="ExternalOutput")
    cnew_d = nc.dram_tensor("cnew", [K, D], f32, kind="ExternalOutput")
    mnew_d = nc.dram_tensor("mnew", [K, D], f32, kind="ExternalOutput")
    if DBG:
        dbg_ws = nc.dram_tensor("dbg_ws", [K, D], f32, kind="ExternalOutput")
        dbg_sumw = nc.dram_tensor("dbg_sumw", [128, KC], f32, kind="ExternalOutput")
        dbg_rep = nc.dram_tensor("dbg_rep", [K, D], f32, kind="ExternalOutput")

    with tile.TileContext(nc) as tc:
        with tc.tile_pool(name="const", bufs=1) as const, \
             tc.tile_pool(name="dram", bufs=1, space="DRAM") as dram:
            # ---------------- constants ----------------
            ones_stage = const.tile([128, 512], f32)
            nc.gpsimd.memset(ones_stage[:], 1.0)
            ones_col = const.tile([128, 1], f32r)
            nc.vector.tensor_copy(ones_col[:], ones_stage[:, 0:1])
            ones_row = const.tile([1, 128], f32r)
            nc.vector.tensor_copy(ones_row[:], ones_stage[0:1, 0:128])
            ones_row512 = const.tile([1, 512], f32r)
            nc.vector.tensor_copy(ones_row512[:], ones_stage[0:1, :])
            eps_col = const.tile([128, 1], f32)
            nc.gpsimd.memset(eps_col[:], EPS)

            cT_r = const.tile([128, DC, K], f32r)     # c^T: [d-part, dchunk, k]
            nc.sync.dma_start(cT_r[:], cT_d[:].rearrange("(i p) k -> p i k", p=128))
            nhcsq_row = const.tile([1, K], f32r)      # -|c_k|^2 / 2 as a row
            nc.sync.dma_start(nhcsq_row[:], nhcsq_d[:])

            c_sb = const.tile([128, KC, D], f32)      # c rows, chunked
            nc.sync.dma_start(c_sb[:], c_d[:].rearrange("(j p) d -> p j d", p=128))
            c_r = const.tile([128, KC, D], f32r)      # f32r-rounded c rows
            nc.vector.tensor_copy(c_r[:], c_sb[:])

            srep = const.tile([128, KC, K], f32r, name="srep")
            s0 = const.tile([128, KC], f32, name="s0")
            cmr = const.tile([128, KC, D], f32, name="cmr")     # c - rep
            mom9 = const.tile([128, KC, D], f32, name="mom9")   # 0.9 * mom

            # ---------------- repulsion emitters ----------------
            def emit_rep_phase1(rps):
                """G, sq, dist, srep, s0 - before the loop's softmax so the
                ACT table switches only once (Sqrt set -> Exp set)."""
                for I in range(KC):
                    g_ps = rps.tile([128, K], f32, name=f"g_ps{I}", tag="rep_ps")
                    for i in range(DC):
                        nc.tensor.matmul(
                            g_ps[:], cT_r[:, i, I * 128:(I + 1) * 128],
                            cT_r[:, i, :], start=(i == 0), stop=False)
                    nc.tensor.matmul(g_ps[:], ones_row[:], nhcsq_row[:],
                                     start=False, stop=False)
                    nc.tensor.matmul(g_ps[:],
                                     nhcsq_row[:, I * 128:(I + 1) * 128],
                                     ones_row512[:], start=False, stop=True)
                    # sq = max(-2 * psum, 0)  (psum = G - csq_i/2 - csq_j/2;
                    # the clamp keeps the diagonal from going negative)
                    sq = const.tile([128, K], f32, name=f"sq{I}", tag="sq")
                    nc.vector.tensor_scalar(sq[:], g_ps[:], -2.0, 0.0,
                                            op0=OP.mult, op1=OP.max)
                    dist = const.tile([128, K], f32, name=f"dist{I}", tag="dist")
                    nc.scalar.activation(dist[:], sq[:], AF.Sqrt, bias=eps_col[:])
                    dpe = const.tile([128, K], f32, name=f"dpe{I}", tag="dpe")
                    nc.vector.tensor_scalar_add(dpe[:], dist[:], EPS)
                    rcp = const.tile([128, K], f32, name=f"rcp{I}", tag="rcp")
                    nc.vector.reciprocal(rcp[:], dpe[:])
                    w = const.tile([128, K], f32, name=f"w{I}", tag="w")
                    nc.vector.tensor_scalar(w[:], dist[:], -1.0, 1.0,
                                            op0=OP.mult, op1=OP.add)
                    nc.vector.tensor_scalar_max(w[:], w[:], 0.0)
                    nc.vector.tensor_mul(w[:], w[:], rcp[:])
                    nc.vector.tensor_scalar(srep[:, I, :], w[:], REP_STRENGTH,
                                            None, op0=OP.mult)
                    # s0 must sum the *rounded* srep values so the huge diagonal
                    # cancels exactly against S @ c
                    nc.vector.tensor_reduce(s0[:, I:I + 1], srep[:, I, :],
                                            axis=AX.X, op=OP.add)

            def emit_rep_phase2_chunk(rps, I):
                """S @ c chunk I and the rep combine into cmr (S symmetric,
                so lhsT slices come straight from srep rows)."""
                sc_ps = rps.tile([128, D], f32, name=f"sc_ps{I}", tag="rep_ps")
                for J in range(KC):
                    nc.tensor.matmul(
                        sc_ps[:], srep[:, J, I * 128:(I + 1) * 128],
                        c_r[:, J, :], start=(J == 0), stop=(J == KC - 1))
                # rep = s0 * c_r - S @ c_r (same rounded c on both sides so the
                # huge diagonal term cancels exactly); cmr = c - rep
                nc.vector.scalar_tensor_tensor(
                    cmr[:, I, :], c_r[:, I, :], s0[:, I:I + 1],
                    sc_ps[:], op0=OP.mult, op1=OP.subtract)
                if DBG:
                    nc.sync.dma_start(
                        dbg_rep[I * 128:(I + 1) * 128, :], cmr[:, I, :])
                nc.vector.tensor_sub(cmr[:, I, :], c_sb[:, I, :], cmr[:, I, :])

            # ---------------- main loop (pipelined emission) ----------------
            with tc.tile_pool(name="ws_pool", bufs=1, space="PSUM") as wps, \
                 tc.tile_pool(name="sumw_pool", bufs=1, space="PSUM") as swps:
                ws_ps = [wps.tile([128, D], f32, name=f"ws_ps{j}") for j in range(KC)]
                sumw_ps = swps.tile([1, K], f32, name="sumw_ps")

                with tc.tile_pool(name="xt", bufs=1) as xtp, \
                     tc.tile_pool(name="work", bufs=3) as wk, \
                     tc.tile_pool(name="rp_ps", bufs=1, space="PSUM") as rps, \
                     tc.tile_pool(name="cr_ps", bufs=2, space="PSUM") as cps:
                    xhT = {}
                    xlT = {}
                    xrT = {}
                    a_tiles = {}
                    xr_tiles = {}

                    def emit_transposes(g):
                        r0 = g * GROUP
                        xhT[g] = xtp.tile([128, DC, GROUP], bf16,
                                          name=f"xhT{g}", tag="xhT", bufs=NGRP)
                        xlT[g] = xtp.tile([128, DC, GROUP], bf16,
                                          name=f"xlT{g}", tag="xlT", bufs=NGRP)
                        xrT[g] = xtp.tile([128, DC, GROUP], f32r,
                                          name=f"xrT{g}", tag="xrT", bufs=4)
                        for i in range(DC):
                            # xbar transposes must stay on their own HWDGE ring
                            # (ACT): mixing with plain copies on one ring hits
                            # the DMATranspose<->DMACopy xbar-mode HW bug.
                            nc.scalar.dma_start(
                                xhT[g][:, i, :],
                                xh_d[r0:r0 + GROUP, i * 128:(i + 1) * 128],
                                transpose=True)
                            nc.scalar.dma_start(
                                xlT[g][:, i, :],
                                xl_d[r0:r0 + GROUP, i * 128:(i + 1) * 128],
                                transpose=True)
                            nc.vector.tensor_add(xrT[g][:, i, :], xhT[g][:, i, :],
                                                 xlT[g][:, i, :])

                    def emit_cross(n):
                        g, t = n // TPG, n % TPG
                        n0 = n * 128
                        xh_n = wk.tile([128, D], bf16, name=f"xh_n{n}",
                                       tag="xh_n", bufs=4)
                        xl_n = wk.tile([128, D], bf16, name=f"xl_n{n}",
                                       tag="xl_n", bufs=4)
                        nc.sync.dma_start(xh_n[:], xh_d[n0:n0 + 128, :])
                        nc.sync.dma_start(xl_n[:], xl_d[n0:n0 + 128, :])
                        xr_n = wk.tile([128, D], f32r, name=f"xr_n{n}",
                                       tag="xr_n", bufs=6)
                        nc.vector.tensor_add(xr_n[:], xh_n[:], xl_n[:])
                        xr_tiles[n] = xr_n

                        ps_c = cps.tile([128, K], f32, name=f"ps_c{n}", tag="ps_c")
                        for i in range(DC):
                            nc.tensor.matmul(
                                ps_c[:], xrT[g][:, i, t * 128:(t + 1) * 128],
                                cT_r[:, i, :], start=(i == 0), stop=False)
                        nc.tensor.matmul(ps_c[:], ones_row[:], nhcsq_row[:],
                                         start=False, stop=True)

                        mx = wk.tile([128, 1], f32, name=f"mx{n}", tag="mx")
                        nc.vector.tensor_reduce(mx[:], ps_c[:], axis=AX.X, op=OP.max)
                        nbias = wk.tile([128, 1], f32, name=f"nb{n}", tag="nb")
                        nc.vector.tensor_scalar(nbias[:], mx[:], -2.0 / TEMP, None,
                                                op0=OP.mult)
                        a_un = wk.tile([128, K], f32, name=f"a_un{n}", tag="a_un")
                        sume = wk.tile([128, 1], f32, name=f"sume{n}", tag="sume")
                        nc.scalar.activation(a_un[:], ps_c[:], AF.Exp,
                                             bias=nbias[:], scale=2.0 / TEMP,
                                             accum_out=sume[:])
                        den = wk.tile([128, 1], f32, name=f"den{n}", tag="den")
                        nc.vector.tensor_scalar_add(den[:], sume[:], EPS)
                        rcp_n = wk.tile([128, 1], f32, name=f"rcp_n{n}", tag="rcp_n")
                        nc.vector.reciprocal(rcp_n[:], den[:])
                        a_sb = wk.tile([128, K], f32r, name=f"a_sb{n}",
                                       tag="a_sb", bufs=4)
                        nc.vector.tensor_scalar(a_sb[:], a_un[:], rcp_n[:], None,
                                                op0=OP.mult)
                        nc.gpsimd.dma_start(a_d[n0:n0 + 128, :],
                                            a_sb[:].bitcast(mybir.dt.float32))
                        a_tiles[n] = a_sb

                    def emit_ws(n):
                        a_sb = a_tiles.pop(n)
                        xr_n = xr_tiles.pop(n)
                        for j in range(KC):
                            nc.tensor.matmul(
                                ws_ps[j][:], a_sb[:, j * 128:(j + 1) * 128],
                                xr_n[:], start=(n == 0), stop=(n == NT - 1))
                        nc.tensor.matmul(sumw_ps[:], ones_col[:], a_sb[:],
                                         start=(n == 0), stop=(n == NT - 1))

                    emit_transposes(0)
                    emit_transposes(1)
                    emit_rep_phase1(rps)
                    for n in range(NT + 1):
                        if n < NT:
                            if n % TPG == 0 and (g2 := n // TPG + 2) < NGRP:
                                emit_transposes(g2)
                            emit_cross(n)
                        if 8 <= n < 8 + KC:
                            emit_rep_phase2_chunk(rps, n - 8)
                        if n == 16:
                            # 0.9 * mom staged during the loop (off crit path)
                            nc.sync.dma_start(
                                mom9[:],
                                mom_d[:].rearrange("(j p) d -> p j d", p=128))
                            nc.vector.tensor_scalar(mom9[:], mom9[:], MOMENTUM,
                                                    None, op0=OP.mult)
                        if n >= 1:
                            emit_ws(n - 1)

                # -------- drain partials --------
                ws_sb, _free_ws_sb = tc.tile([128, KC, D], f32, name="ws_sb")
                for j in range(KC):
                    nc.vector.tensor_copy(ws_sb[:, j, :], ws_ps[j][:])
                sumw_sb, _free_sumw_sb = tc.tile([1, K], f32, name="sumw_sb")
                nc.vector.tensor_copy(sumw_sb[:], sumw_ps[:])

            ar_in = dram.tile([K + 1, D], mybir.dt.float32, name="ar_in")
            ar_out = dram.tile([K + 1, D], mybir.dt.float32, name="ar_out",
                               addr_space="Shared")
            nc.sync.dma_start(
                ar_in[0:K, :].rearrange("(j p) d -> p j d", p=128), ws_sb[:])
            nc.sync.dma_start(ar_in[K:K + 1, :], sumw_sb[:])
            if TLSIM:
                # cost-model build: TimelineSim cannot model collectives
                nc.sync.dma_start(ar_out[:], ar_in[:])
            else:
                nc.gpsimd.collective_compute(
                    "AllReduce",
                    mybir.AluOpType.add,
                    replica_groups=[list(range(N_CORES))],
                    ins=[ar_in[:].opt()],
                    outs=[ar_out[:].opt()],
                )

            # -------- final update (redundant on every core) --------
            with tc.tile_pool(name="tail_ps", bufs=1, space="PSUM") as tps:
                # reuse ws_sb as the post-allreduce buffer
                nc.sync.dma_start(ws_sb[:],
                                  ar_out[0:K, :].rearrange("(j p) d -> p j d", p=128))
                sumw_row, _free_sumw_row = tc.tile([1, K], f32, name="sumw_row")
                nc.sync.dma_start(sumw_row[:], ar_out[K:K + 1, :])
                # transpose the [1, 512] row into [128, 4] via PE
                swt_ps = tps.tile([128, KC], f32, name="swt_ps")
                for j in range(KC):
                    nc.tensor.transpose(swt_ps[:, j:j + 1],
                                        sumw_row[:, j * 128:(j + 1) * 128],
                                        ones_stage[0:1, 0:1])
                sumw_col, _free_sumw_col = tc.tile([128, KC], f32, name="sumw_col")
                nc.vector.tensor_scalar_add(sumw_col[:], swt_ps[:], EPS)
                rcp_c, _free_rcp_c = tc.tile([128, KC], f32, name="rcp_c")
                nc.vector.reciprocal(rcp_c[:], sumw_col[:])

                if DBG:
                    nc.sync.dma_start(dbg_ws[:].rearrange("(j p) d -> p j d", p=128),
                                      ws_sb[:])
                    nc.sync.dma_start(dbg_sumw[:], sumw_col[:])

                cn_sb, _free_cn_sb = tc.tile([128, KC, D], f32, name="cn_sb")
                mn_sb, _free_mn_sb = tc.tile([128, KC, D], f32, name="mn_sb")
                for j in range(KC):
                    # u = ws * rcp - (c - rep)   (into cn_sb as scratch)
                    nc.vector.tensor_scalar(cn_sb[:, j, :], ws_sb[:, j, :],
                                            rcp_c[:, j:j + 1], None, op0=OP.mult)
                    nc.vector.tensor_sub(cn_sb[:, j, :], cn_sb[:, j, :],
                                         cmr[:, j, :])
                    # mom_new = 0.9*mom + 0.1*u
                    nc.vector.scalar_tensor_tensor(
                        mn_sb[:, j, :], cn_sb[:, j, :], 1.0 - MOMENTUM,
                        mom9[:, j, :], op0=OP.mult, op1=OP.add)
                    nc.sync.dma_start(mnew_d[j * 128:(j + 1) * 128, :],
                                      mn_sb[:, j, :])
                    # c_new = c + lr * mom_new
                    nc.vector.scalar_tensor_tensor(
                        cn_sb[:, j, :], mn_sb[:, j, :], CENTROID_LR,
                        c_sb[:, j, :], op0=OP.mult, op1=OP.add)
                    nc.sync.dma_start(cnew_d[j * 128:(j + 1) * 128, :],
                                      cn_sb[:, j, :])

            for _f in (_free_mn_sb, _free_cn_sb, _free_rcp_c, _free_sumw_col,
                       _free_sumw_row, _free_sumw_sb, _free_ws_sb):
                _f()

    nc.finalize()
    return nc


def _f32r_round(a):
    """Round-to-nearest-even fp32 -> fp32r (1s + 8e + 11 explicit mantissa)."""
    v = np.ascontiguousarray(a, dtype=np.float32).view(np.uint32)
    r = v + 0x7FF + ((v >> 12) & 1)
    r &= np.uint32(0xFFFFF000)
    return r.view(np.float32)


def _prep_inputs(x, centroids, centroid_momentum):
    bf16 = ml_dtypes.bfloat16
    x = np.ascontiguousarray(x, dtype=np.float32).reshape(N_CORES, NLOC, D)
    xh = x.astype(bf16)
    xl = (x - xh.astype(np.float32)).astype(bf16)
    c = np.ascontiguousarray(centroids, dtype=np.float32)
    c_r = _f32r_round(c)
    cT = np.ascontiguousarray(c_r.T)
    nhcsq = _f32r_round(
        (-0.5 * (c_r.astype(np.float64) ** 2).sum(axis=1)).astype(np.float32)
    ).reshape(1, K)
    m = np.ascontiguousarray(centroid_momentum, dtype=np.float32)
    in_maps = []
    for i in range(N_CORES):
        in_maps.append({
            "xh": np.ascontiguousarray(xh[i]),
            "xl": np.ascontiguousarray(xl[i]),
            "c": c,
            "cT": cT,
            "nhcsq": nhcsq,
            "mom": m,
        })
    return in_maps


def kernel(x, centroids, centroid_momentum, _trace=False):
    import os
    from concourse import bass_utils

    if "nc" not in _CACHE:
        _CACHE["nc"] = _build_nc()
    nc = _CACHE["nc"]

    in_maps = _prep_inputs(x, centroids, centroid_momentum)
    env_backup = os.environ.get("BASS_NEVER_TRACE")
    if not _trace:
        # the axon NTFF hook is not importable in this container; force the
        # no-trace path even if BASS_TRACE is set globally
        os.environ["BASS_NEVER_TRACE"] = "1"
    try:
        res = bass_utils.run_bass_kernel_spmd(
            nc, in_maps, core_ids=list(range(N_CORES)), trace=_trace)
    finally:
        if not _trace:
            if env_backup is None:
                os.environ.pop("BASS_NEVER_TRACE", None)
            else:
                os.environ["BASS_NEVER_TRACE"] = env_backup
    _CACHE["last_result"] = res

    a = np.stack([res.results[i]["a_out"] for i in range(N_CORES)], axis=0)
    a = a.reshape(B, S, K)
    cnew = res.results[0]["cnew"]
    mnew = res.results[0]["mnew"]
    return a, cnew, mnew
